# revision 2
# baseline (speedup 1.0000x reference)
"""Trainium2 Bass kernel for nn_DepthLossV2 (N=8192 pairwise depth loss).

Math: with p = predictions[:,0], s = STEP*z_spacing*nth_slice,
  steps[i,j] = |i-j|*s,  a[i,j] = p[i]-p[j]
  d = where(a>=0, a-0.2*steps, a); d = where(d>=0, max(d-0.8*steps,0), d)
  loss = sum(|tril(d)|)/N^2
Closed form of the summand (u = 0.2*s*|i-j|, valid for s >= 0):
  f(a,u) = relu(max(a - 5u, u*[a>=0] - a))
which is computed by ONE custom DVE op per tile (7 ALU stages + ADD
accumulation into a per-partition accumulator), with
  a  : from the TensorEngine via a K=2 matmul  [-1; p_i]^T @ [p_j; 1] -> PSUM
  u  : from the ScalarEngine via Abs(0.2*s*j - 0.2*s*i) with per-partition bias

Sharding: 64 row-tiles of 128 rows. Core c, slot t handles global row-tile
g = 8t + c over columns [0, 1024*(t+1)) — a superset of the tril extent that is
shape-uniform across cores (SPMD: one program, per-core data). Every core does
exactly 36864 column-elements of DVE work. The over-computed wedge
(j > i, j < 1024*(t+1)) is subtracted on the host in float64.
"""

import os

import numpy as np

N = 8192
P = 128
NCORES = 8
SLOTS = 8
STEP = 1.0

_CACHE = {}
last_exec_ns = None
last_trace = None


def _register_depth_op():
    import concourse.dve_ops as dve_ops
    from concourse.dve_ops import DveOp, OPS
    from concourse.dve_spec import (
        Spec, Src0, Src1, C1, Zero, AluOp, lower, maxx, relu, _has_src1,
    )
    from concourse.dve_uop import DveOpSpec

    name = "DEPTHLOSS_F_ANT"
    if name in dve_ops._SUB_OPCODE_FOR_NAME:
        return next(op for op in OPS if op.name == name)

    # in0 = a (PSUM), in1 = u (SBUF), s1 = C1 = 5.0
    # out = relu(max(a - 5u, u*[a>=0] - a)); accum_out = sum(out)
    m = Src0 >= Zero
    w = Src1 * m - Src0
    v = Src0 - Src1 * C1
    body = relu(maxx(v, w))

    def ref(in0, in1, s0, s1, imm2):
        mm = (in0 >= 0).astype(in0.dtype)
        out = np.maximum(np.maximum(in0 - in1 * s1, in1 * mm - in0), 0.0)
        return out, out.sum(axis=-1, keepdims=True)

    spec = Spec(body=body, accum=AluOp.ADD, reference=ref)
    row = dve_ops._CUSTOM_DVE_ROW_BASE + len(OPS)
    assert row < 0x20, "no free custom-DVE opcode rows"
    shas = {}
    for ver in ("v3", "v4"):
        d = DveOpSpec(name=name, opcode=row, uops=lower(spec, ver=ver),
                      rd1_en=_has_src1(spec))
        shas[ver] = d.sha(ver)
    op = DveOp(name, spec, subdim=False, uops_sha=shas)
    OPS.append(op)
    dve_ops._SUB_OPCODE_FOR_NAME[name] = row
    dve_ops.CUSTOM_DVE_SPECS[name] = spec
    return op


def _chunks_for_slot(t):
    """(col_offset, width) chunks covering [0, 1024*(t+1)), widths 2048/1024."""
    total = 1024 * (t + 1)
    out = []
    c0 = 0
    while total - c0 >= 2048:
        out.append((c0, 2048))
        c0 += 2048
    if c0 < total:
        out.append((c0, total - c0))
    return out


def _n_units():
    return sum(len(_chunks_for_slot(t)) for t in range(SLOTS))


def _build_program(scale02):
    """Build + Bacc-compile the SPMD program for one core. scale02 = 0.2*s."""
    import concourse.bacc as bacc
    import concourse.mybir as mybir
    import concourse.tile as tile

    depth_op = _register_depth_op()

    nunits = _n_units()
    nc = bacc.Bacc(trn_type="TRN2", name="depthloss")
    mat_d = nc.dram_tensor("mat", [2, N + SLOTS * P], mybir.dt.float32,
                           kind="ExternalInput")
    bias_d = nc.dram_tensor("bias", [P, SLOTS], mybir.dt.float32,
                            kind="ExternalInput")
    acc_d = nc.dram_tensor("acc", [P, nunits], mybir.dt.float32,
                           kind="ExternalOutput")

    with tile.TileContext(nc) as tc:
        with (
            tc.tile_pool(name="persist", bufs=1) as persist,
            tc.tile_pool(name="psum", bufs=2, space="PSUM") as psum,
            tc.tile_pool(name="work", bufs=3) as work,
        ):
            mat_t = persist.tile([2, N + SLOTS * P], mybir.dt.float32)
            nc.sync.dma_start(mat_t[:], mat_d[:])
            bias_t = persist.tile([P, SLOTS], mybir.dt.float32)
            nc.sync.dma_start(bias_t[:], bias_d[:])

            jota = persist.tile([P, N], mybir.dt.float32)
            nc.gpsimd.iota(jota[:], pattern=[[1, N]], base=0,
                           channel_multiplier=0,
                           allow_small_or_imprecise_dtypes=True)

            acc_t = persist.tile([P, nunits], mybir.dt.float32)

            unit = 0
            for t in range(SLOTS):
                lhs = mat_t[:, N + t * P:N + (t + 1) * P]
                for (c0, cw) in _chunks_for_slot(t):
                    a_ps = psum.tile([P, 2048], mybir.dt.float32, tag="a")
                    for k in range(cw // 512):
                        nc.tensor.matmul(
                            a_ps[:, k * 512:(k + 1) * 512], lhs,
                            mat_t[:, c0 + k * 512:c0 + (k + 1) * 512],
                            start=True, stop=True)
                    u_t = work.tile([P, 2048], mybir.dt.float32, tag="u")
                    nc.scalar.activation(
                        u_t[:, :cw], jota[:, c0:c0 + cw],
                        mybir.ActivationFunctionType.Abs,
                        bias=bias_t[:, t:t + 1], scale=scale02)
                    f_t = work.tile([P, 2048], mybir.dt.float32, tag="f")
                    nc.vector._custom_dve(
                        depth_op, out=f_t[:, :cw], in0=a_ps[:, :cw],
                        in1=u_t[:, :cw], s1=5.0,
                        accum_out=acc_t[:, unit:unit + 1])
                    unit += 1

            nc.sync.dma_start(acc_d[:], acc_t[:])

    nc.compile()
    return nc, nunits


def _host_f(a, u):
    return np.maximum(np.maximum(a - 5.0 * u, u * (a >= 0) - a), 0.0)


def _wedge_correction(p64, scale02):
    """Sum of f over the over-computed region (j > i) in float64."""
    corr = 0.0
    for t in range(SLOTS):
        jmax = 1024 * (t + 1)
        for c in range(NCORES):
            g = SLOTS * t + c
            i = np.arange(P * g, P * g + P, dtype=np.float64)
            j = np.arange(P * g, jmax, dtype=np.float64)
            if j.size == 0:
                continue
            a = p64[i.astype(int)][:, None] - p64[None, j.astype(int)]
            u = scale02 * np.abs(i[:, None] - j[None, :])
            f = _host_f(a, u)
            corr += f[j[None, :] > i[:, None]].sum()
    return corr


def kernel(predictions, z_spacing, nth_slice):
    global last_exec_ns, last_trace
    p = np.asarray(predictions, dtype=np.float32).reshape(N)
    s = float(STEP) * float(np.asarray(z_spacing)) * float(np.asarray(nth_slice))

    if not (s >= 0.0) or not np.isfinite(s):
        # negative/NaN step never occurs with the reference setup; fall back
        # to exact host evaluation for robustness.
        p64 = p.astype(np.float64)
        i = np.arange(N, dtype=np.float64)
        st = np.abs(i[:, None] - i[None, :]) * s
        a = p64[:, None] - p64[None, :]
        d = np.where(a >= 0, a - 0.2 * st, a)
        d = np.where(d >= 0, np.maximum(d - 0.8 * st, 0.0), d)
        return np.float32(np.abs(np.tril(d)).sum() / (N * N))

    scale02 = 0.2 * s
    key = np.float32(scale02).item()
    if key not in _CACHE:
        _CACHE[key] = _build_program(np.float32(scale02).item())
    nc, nunits = _CACHE[key]

    # per-core inputs
    in_maps = []
    for c in range(NCORES):
        mat = np.empty((2, N + SLOTS * P), np.float32)
        mat[0, :N] = p
        mat[1, :N] = 1.0
        bias = np.empty((P, SLOTS), np.float32)
        for t in range(SLOTS):
            g = SLOTS * t + c
            rows = slice(P * g, P * g + P)
            mat[0, N + t * P:N + (t + 1) * P] = -1.0
            mat[1, N + t * P:N + (t + 1) * P] = p[rows]
            bias[:, t] = -scale02 * np.arange(P * g, P * g + P, dtype=np.float32)
        in_maps.append({"mat": mat, "bias": bias})

    from concourse.bass_utils import run_bass_kernel_spmd
    trace = bool(int(os.environ.get("DEPTH_TRACE", "0")))
    if trace:
        try:
            import antenv.axon_hooks  # noqa: F401
        except ImportError:
            trace = False
    res = run_bass_kernel_spmd(nc, in_maps, core_ids=list(range(NCORES)),
                               trace=trace)
    last_exec_ns = res.exec_time_ns
    last_trace = res.instructions_and_trace
    total = np.float64(0.0)
    for r in res.results:
        total += r["acc"].astype(np.float64).sum()

    corr = _wedge_correction(p.astype(np.float64), np.float64(scale02))
    loss = (total - corr) / (N * N)
    return np.float32(loss)


# revision 9
# speedup vs baseline: 1.9675x; 1.9675x over previous
"""Trainium2 Bass kernel for nn_DepthLossV2 (N=8192 pairwise depth loss).

Math: with p = predictions[:,0], s = STEP*z_spacing*nth_slice,
  steps[i,j] = |i-j|*s,  a[i,j] = p[i]-p[j]
  d = where(a>=0, a-0.2*steps, a); d = where(d>=0, max(d-0.8*steps,0), d)
  loss = sum(|tril(d)|)/N^2
Closed form of the summand (u = 0.2*s*|i-j|, valid for s >= 0):
  f(a,u) = relu(max(a - 5u, u*[a>=0] - a))
which is computed by ONE custom DVE op per tile (7 ALU stages + ADD
accumulation into a per-partition accumulator), with
  a  : from the TensorEngine via a K=2 matmul  [-1; p_i]^T @ [p_j; 1] -> PSUM
  u  : from the ScalarEngine via Abs(0.2*s*j - 0.2*s*i) with per-partition bias

Sharding: 64 row-tiles of 128 rows. Core c, slot t handles global row-tile
g = 8t + c over columns [0, 1024*(t+1)) — a superset of the tril extent that is
shape-uniform across cores (SPMD: one program, per-core data). Every core does
exactly 36864 column-elements of DVE work. The over-computed wedge
(j > i, j < 1024*(t+1)) is subtracted on the host in float64.
"""

import os

import numpy as np

N = 8192
P = 128
NCORES = 8
SLOTS = 8
STEP = 1.0

_CACHE = {}
last_exec_ns = None
last_trace = None


def _register_depth_op():
    import concourse.dve_ops as dve_ops
    from concourse.dve_ops import DveOp, OPS
    from concourse.dve_spec import (
        Spec, Src0, Src1, C1, Zero, AluOp, lower, maxx, relu, _has_src1,
    )
    from concourse.dve_uop import DveOpSpec

    name = "DEPTHLOSS_F_ANT"
    if name in dve_ops._SUB_OPCODE_FOR_NAME:
        return next(op for op in OPS if op.name == name)

    # in0 = a (PSUM), in1 = u (SBUF), s1 = C1 = 5.0
    # out = relu(max(a - 5u, u*[a>=0] - a)); accum_out = sum(out)
    m = Src0 >= Zero
    w = Src1 * m - Src0
    v = Src0 - Src1 * C1
    body = relu(maxx(v, w))

    def ref(in0, in1, s0, s1, imm2):
        mm = (in0 >= 0).astype(in0.dtype)
        out = np.maximum(np.maximum(in0 - in1 * s1, in1 * mm - in0), 0.0)
        return out, out.sum(axis=-1, keepdims=True)

    spec = Spec(body=body, accum=AluOp.ADD, reference=ref)
    row = dve_ops._CUSTOM_DVE_ROW_BASE + len(OPS)
    assert row < 0x20, "no free custom-DVE opcode rows"
    shas = {}
    for ver in ("v3", "v4"):
        d = DveOpSpec(name=name, opcode=row, uops=lower(spec, ver=ver),
                      rd1_en=_has_src1(spec))
        shas[ver] = d.sha(ver)
    op = DveOp(name, spec, subdim=False, uops_sha=shas)
    OPS.append(op)
    dve_ops._SUB_OPCODE_FOR_NAME[name] = row
    dve_ops.CUSTOM_DVE_SPECS[name] = spec
    return op


def _chunks_for_slot(t):
    """(col_offset, width) chunks covering [0, 1024*(t+1)), widths 2048/1024."""
    total = 1024 * (t + 1)
    out = []
    c0 = 0
    while total - c0 >= 2048:
        out.append((c0, 2048))
        c0 += 2048
    if c0 < total:
        out.append((c0, total - c0))
    return out


def _n_units():
    return sum(len(_chunks_for_slot(t)) for t in range(SLOTS))


def _build_program(scale02):
    """Build + Bacc-compile the SPMD program for one core. scale02 = 0.2*s."""
    import concourse.bacc as bacc
    import concourse.mybir as mybir
    import concourse.tile as tile

    depth_op = _register_depth_op()

    nunits = _n_units()
    nc = bacc.Bacc(trn_type="TRN2", name="depthloss")
    mat_d = nc.dram_tensor("mat", [4, N + SLOTS * P], mybir.dt.bfloat16,
                           kind="ExternalInput")
    bias_d = nc.dram_tensor("bias", [P, SLOTS], mybir.dt.float32,
                            kind="ExternalInput")
    acc_d = nc.dram_tensor("acc", [P, nunits], mybir.dt.float32,
                           kind="ExternalOutput")

    with tile.TileContext(nc) as tc:
        with (
            tc.tile_pool(name="persist", bufs=1) as persist,
            tc.tile_pool(name="psum", bufs=2, space="PSUM") as psum,
            tc.tile_pool(name="upool", bufs=6) as upool,
            tc.tile_pool(name="work", bufs=2) as work,
        ):
            jota = persist.tile([P, N], mybir.dt.float32)
            bounds = [0, 1024, 2048, 4096, N]
            for q in range(4):
                b0, b1 = bounds[q], bounds[q + 1]
                nc.gpsimd.iota(jota[:, b0:b1], pattern=[[1, b1 - b0]], base=b0,
                               channel_multiplier=0,
                               allow_small_or_imprecise_dtypes=True)

            mat_t = persist.tile([4, N + SLOTS * P], mybir.dt.bfloat16)
            nc.sync.dma_start(mat_t[:], mat_d[:])
            bias_t = persist.tile([P, SLOTS], mybir.dt.float32)
            nc.sync.dma_start(bias_t[:], bias_d[:])

            # warm the ACT function table off the critical path
            warm_t = work.tile([P, 1], mybir.dt.float32, tag="warm")
            nc.scalar.activation(warm_t[:], bias_t[:, 0:1],
                                 mybir.ActivationFunctionType.Abs,
                                 bias=0.0, scale=1.0)

            acc_t = persist.tile([P, nunits], mybir.dt.float32)

            unit = 0
            for t in range(SLOTS):
                lhs = mat_t[:, N + t * P:N + (t + 1) * P]
                for (c0, cw) in _chunks_for_slot(t):
                    a_ps = psum.tile([P, 2048], mybir.dt.float32, tag="a")
                    for k in range(cw // 512):
                        nc.tensor.matmul(
                            a_ps[:, k * 512:(k + 1) * 512], lhs,
                            mat_t[:, c0 + k * 512:c0 + (k + 1) * 512],
                            start=True, stop=True)
                    u_t = upool.tile([P, 2048], mybir.dt.float32, tag="u")
                    nc.scalar.activation(
                        u_t[:, :cw], jota[:, c0:c0 + cw],
                        mybir.ActivationFunctionType.Abs,
                        bias=bias_t[:, t:t + 1], scale=scale02)
                    f_t = work.tile([P, 2048], mybir.dt.float32, tag="f")
                    nc.vector._custom_dve(
                        depth_op, out=f_t[:, :cw], in0=a_ps[:, :cw],
                        in1=u_t[:, :cw], s1=5.0,
                        accum_out=acc_t[:, unit:unit + 1])
                    unit += 1

            nc.sync.dma_start(acc_d[:], acc_t[:])

    nc.compile()
    return nc, nunits


def _host_f(a, u):
    return np.maximum(np.maximum(a - 5.0 * u, u * (a >= 0) - a), 0.0)


def _wedge_correction(p64, scale02):
    """Sum of f over the over-computed region (j > i) in float64."""
    corr = 0.0
    for t in range(SLOTS):
        jmax = 1024 * (t + 1)
        for c in range(NCORES):
            g = SLOTS * t + c
            i = np.arange(P * g, P * g + P, dtype=np.float64)
            j = np.arange(P * g, jmax, dtype=np.float64)
            if j.size == 0:
                continue
            a = p64[i.astype(int)][:, None] - p64[None, j.astype(int)]
            u = scale02 * np.abs(i[:, None] - j[None, :])
            f = _host_f(a, u)
            corr += f[j[None, :] > i[:, None]].sum()
    return corr


def kernel(predictions, z_spacing, nth_slice):
    global last_exec_ns, last_trace
    p = np.asarray(predictions, dtype=np.float32).reshape(N)
    s = float(STEP) * float(np.asarray(z_spacing)) * float(np.asarray(nth_slice))

    if not (s >= 0.0) or not np.isfinite(s):
        # negative/NaN step never occurs with the reference setup; fall back
        # to exact host evaluation for robustness.
        p64 = p.astype(np.float64)
        i = np.arange(N, dtype=np.float64)
        st = np.abs(i[:, None] - i[None, :]) * s
        a = p64[:, None] - p64[None, :]
        d = np.where(a >= 0, a - 0.2 * st, a)
        d = np.where(d >= 0, np.maximum(d - 0.8 * st, 0.0), d)
        return np.float32(np.abs(np.tril(d)).sum() / (N * N))

    scale02 = 0.2 * s
    key = np.float32(scale02).item()
    if key not in _CACHE:
        _CACHE[key] = _build_program(np.float32(scale02).item())
    nc, nunits = _CACHE[key]

    # per-core inputs
    in_maps = []
    for c in range(NCORES):
        import ml_dtypes
        p_hi = p.astype(ml_dtypes.bfloat16)
        p_lo = (p - p_hi.astype(np.float32)).astype(ml_dtypes.bfloat16)
        mat = np.empty((4, N + SLOTS * P), ml_dtypes.bfloat16)
        mat[0, :N] = p_hi
        mat[1, :N] = p_lo
        mat[2, :N] = 1.0
        mat[3, :N] = 1.0
        bias = np.empty((P, SLOTS), np.float32)
        for t in range(SLOTS):
            g = SLOTS * t + c
            rows = slice(P * g, P * g + P)
            mat[0, N + t * P:N + (t + 1) * P] = -1.0
            mat[1, N + t * P:N + (t + 1) * P] = -1.0
            mat[2, N + t * P:N + (t + 1) * P] = p_hi[rows]
            mat[3, N + t * P:N + (t + 1) * P] = p_lo[rows]
            bias[:, t] = -scale02 * np.arange(P * g, P * g + P, dtype=np.float32)
        in_maps.append({"mat": mat, "bias": bias})

    from concourse.bass_utils import run_bass_kernel_spmd
    trace = bool(int(os.environ.get("DEPTH_TRACE", "0")))
    if trace:
        try:
            import antenv.axon_hooks  # noqa: F401
        except ImportError:
            trace = False
    res = run_bass_kernel_spmd(nc, in_maps, core_ids=list(range(NCORES)),
                               trace=trace)
    last_exec_ns = res.exec_time_ns
    last_trace = res.instructions_and_trace
    total = np.float64(0.0)
    for r in res.results:
        total += r["acc"].astype(np.float64).sum()

    corr = _wedge_correction(p.astype(np.float64), np.float64(scale02))
    loss = (total - corr) / (N * N)
    return np.float32(loss)
